# revision 3
# baseline (speedup 1.0000x reference)
"""MixedLoraLinear (base GEMM + segment-routed LoRA) on 8 TRN2 NeuronCores.

Strategy
--------
Token-shard across the 8 cores (1024 tokens each); replicate weights.
All routing (segment -> adapter -> scaling) is resolved on the host into a
dense [A*R, T] mask*scale matrix MT, so the device program is data-independent.

Per core we compute out^T [D_OUT, 1024]:
  phase A:  hT[ar, t]   = sum_k WAcat[k, ar] * x[t, k]      (A*R = 128 rows)
            htm         = hT * MT_shard                      (mask+scale, DVE)
  phase B:  for each 128-row output block ob:
              psum[oo, t] = sum_k W[ob*128+oo, k] * x[t, k]  (32 k-steps)
                          + sum_ar WBcat[ar, ob*128+oo] * htm[ar, t]  (1 step)
              out = psum + bias  (ScalarE activation w/ per-partition bias)

All matmuls run as float32r (4-xbus fp32 streaming mode: 1 cycle/row at
free-dim 512 vs 4 cycles/row for plain fp32).  x^T stays SBUF-resident
(16 MB); base_w^T streams through double-buffered 2 MB panels.
"""

import numpy as np
from contextlib import ExitStack

import concourse.bass as bass
import concourse.tile as tile
from concourse import bacc, mybir
from concourse.bass_utils import run_bass_kernel_spmd

T, D_IN, D_OUT, R, A = 8192, 4096, 4096, 16, 8
N_CORES = 8
TOK = T // N_CORES          # 1024 tokens per core
KB = D_IN // 128            # 32 contraction blocks
OB = D_OUT // 128           # 32 output-row blocks
AR = A * R                  # 128 = one partition block
FREE = 512                  # matmul moving free dim (1 PSUM bank of fp32)
TH = TOK // FREE            # 2 token halves per core

F32 = mybir.dt.float32
F32R = mybir.dt.float32r


def _build_nc():
    nc = bacc.Bacc("TRN2", target_bir_lowering=False, debug=False,
                   num_devices=N_CORES)
    xt_d = nc.dram_tensor("xt", [128, KB * TOK], F32R, kind="ExternalInput").ap()
    wt_d = nc.dram_tensor("wt", [OB * 128, KB * 128], F32R, kind="ExternalInput").ap()
    wa_d = nc.dram_tensor("wa", [128, KB * AR], F32R, kind="ExternalInput").ap()
    wb_d = nc.dram_tensor("wb", [AR, D_OUT], F32R, kind="ExternalInput").ap()
    mt_d = nc.dram_tensor("mt", [AR, TOK], F32, kind="ExternalInput").ap()
    b_d = nc.dram_tensor("bias", [128, OB], F32, kind="ExternalInput").ap()
    out_d = nc.dram_tensor("outt", [D_OUT, TOK], F32, kind="ExternalOutput").ap()

    with tile.TileContext(nc) as tc, ExitStack() as ctx:
        const = ctx.enter_context(tc.tile_pool(name="const", bufs=1))
        wt_pool = ctx.enter_context(tc.tile_pool(name="wt", bufs=2))
        wa_pool = ctx.enter_context(tc.tile_pool(name="wa", bufs=2))
        wb_pool = ctx.enter_context(tc.tile_pool(name="wb", bufs=2))
        out_pool = ctx.enter_context(tc.tile_pool(name="ot", bufs=4))
        psum_a = ctx.enter_context(tc.tile_pool(name="pa", bufs=1, space="PSUM"))
        psum_b = ctx.enter_context(tc.tile_pool(name="pb", bufs=4, space="PSUM"))

        xt_sb = const.tile([128, KB * TOK], F32R)     # 128 KB/partition, resident
        mt_sb = const.tile([AR, TOK], F32)
        htm_sb = const.tile([AR, TOK], F32R)
        b_sb = const.tile([128, OB], F32)

        # resident loads (xt in 8 x 2MB chunks so several DMA queues engage)
        n_chunks = 8
        cw = KB * TOK // n_chunks
        for i in range(n_chunks):
            nc.sync.dma_start(xt_sb[:, i * cw:(i + 1) * cw],
                              xt_d[:, i * cw:(i + 1) * cw])
        nc.sync.dma_start(mt_sb[:], mt_d[:, :])
        nc.sync.dma_start(b_sb[:], b_d[:, :])

        # ---- phase A: hT = WAcat^T @ x^T, then mask*scale -> htm ----
        pa = []
        for th in range(TH):
            pa_t = psum_a.tile([128, FREE], F32, tag=f"pa{th}")
            pa.append(pa_t)
        for ko in range(KB):
            wa_t = wa_pool.tile([128, AR], F32R)
            nc.sync.dma_start(wa_t[:], wa_d[:, ko * AR:(ko + 1) * AR])
            lhsT = wa_t[:]
            for th in range(TH):
                rhs = xt_sb[:, ko * TOK + th * FREE: ko * TOK + (th + 1) * FREE]
                nc.tensor.matmul(pa[th][:], lhsT=lhsT, rhs=rhs,
                                 start=(ko == 0), stop=(ko == KB - 1))
        for th in range(TH):
            nc.vector.tensor_mul(htm_sb[:, th * FREE:(th + 1) * FREE],
                                 pa[th][:], mt_sb[:, th * FREE:(th + 1) * FREE])

        # ---- phase B: out^T block-by-block, lora fused into same psum ----
        for ob in range(OB):
            wt_t = wt_pool.tile([128, KB * 128], F32R)
            nc.sync.dma_start(wt_t[:], wt_d[ob * 128:(ob + 1) * 128, :])
            wb_t = wb_pool.tile([AR, 128], F32R)
            nc.sync.dma_start(wb_t[:], wb_d[:, ob * 128:(ob + 1) * 128])
            for th in range(TH):
                pb = psum_b.tile([128, FREE], F32)
                for ko in range(KB):
                    nc.tensor.matmul(
                        pb[:],
                        lhsT=wt_t[:, ko * 128:(ko + 1) * 128],
                        rhs=xt_sb[:, ko * TOK + th * FREE:
                                  ko * TOK + (th + 1) * FREE],
                        start=(ko == 0), stop=False)
                nc.tensor.matmul(
                    pb[:], lhsT=wb_t[:],
                    rhs=htm_sb[:, th * FREE:(th + 1) * FREE],
                    start=False, stop=True)
                ot = out_pool.tile([128, FREE], F32)
                nc.scalar.activation(ot[:], pb[:],
                                     mybir.ActivationFunctionType.Identity,
                                     bias=b_sb[:, ob:ob + 1])
                nc.sync.dma_start(
                    out_d[ob * 128:(ob + 1) * 128, th * FREE:(th + 1) * FREE],
                    ot[:])
    nc.compile()
    return nc


_NC = None


def _get_nc():
    global _NC
    if _NC is None:
        _NC = _build_nc()
    return _NC


def _host_prep(x, base_w, base_b, wa, wb, scaling, segment, lora_ids):
    """Build the per-core input maps (all float32 numpy)."""
    x = np.ascontiguousarray(np.asarray(x, np.float32))
    base_w = np.ascontiguousarray(np.asarray(base_w, np.float32))
    base_b = np.ascontiguousarray(np.asarray(base_b, np.float32))
    wa = np.ascontiguousarray(np.asarray(wa, np.float32))
    wb = np.ascontiguousarray(np.asarray(wb, np.float32))
    scaling = np.asarray(scaling, np.float32)
    segment = np.asarray(segment, np.int64)
    lora_ids = np.asarray(lora_ids, np.int64)

    # routing -> dense mask*scale [A*R, T]
    pos = np.arange(T)
    token_seg = np.clip(np.searchsorted(segment, pos, side="right") - 1, 0, A - 1)
    token_lora = lora_ids[token_seg]                      # [T]
    onehot = (token_lora[None, :] == np.arange(A)[:, None]).astype(np.float32)
    mt_full = np.repeat(onehot * scaling[:, None], R, axis=0)  # [A*R, T]
    mt_full = np.ascontiguousarray(mt_full)

    # weights (shared across cores)
    wt_pre = np.ascontiguousarray(
        base_w.reshape(OB, 128, KB, 128).transpose(0, 3, 2, 1)
        .reshape(OB * 128, KB * 128))
    wa_pre = np.ascontiguousarray(
        wa.transpose(1, 0, 2).reshape(KB, 128, AR).transpose(1, 0, 2)
        .reshape(128, KB * AR))
    wb_pre = np.ascontiguousarray(wb.reshape(AR, D_OUT))
    b_pre = np.ascontiguousarray(base_b.reshape(OB, 128).T)

    in_maps = []
    for c in range(N_CORES):
        xs = x[c * TOK:(c + 1) * TOK]                     # [TOK, D_IN]
        xt_pre = np.ascontiguousarray(
            xs.T.reshape(KB, 128, TOK).transpose(1, 0, 2).reshape(128, KB * TOK))
        in_maps.append({
            "xt": xt_pre,
            "wt": wt_pre,
            "wa": wa_pre,
            "wb": wb_pre,
            "mt": np.ascontiguousarray(mt_full[:, c * TOK:(c + 1) * TOK]),
            "bias": b_pre,
        })
    return in_maps


def kernel(x, base_w, base_b, wa, wb, scaling, segment, lora_ids):
    in_maps = _host_prep(x, base_w, base_b, wa, wb, scaling, segment, lora_ids)
    nc = _get_nc()
    res = run_bass_kernel_spmd(nc, in_maps, core_ids=list(range(N_CORES)))
    parts = [res.results[c]["outt"] for c in range(N_CORES)]   # [D_OUT, TOK] each
    out_t = np.concatenate(parts, axis=1)                      # [D_OUT, T]
    return np.ascontiguousarray(out_t.T)                       # [T, D_OUT]
